# revision 49
# baseline (speedup 1.0000x reference)
"""Trainium2 Bass kernel for soft-MoE routing:
    gatings = softmax(x @ gw + gb, axis=1)            # [B, L]
    proj    = einsum('bi,oil->bol', x, pw)            # [B, D_OUT, L]
    result  = einsum('bol,bl->bo', proj, gatings) + gatings @ pb.T

Strategy (data-parallel over batch, 8 NeuronCores, 512 rows each):
  result[b,o] = ( sum_l E[b,l] * (x @ W_l^T + pb_l)[b,o] ) / sum_l E[b,l]
  with E = exp(x@gw + gb) (unnormalized; normalization folded into a final
  elementwise scale). Per core, everything is computed transposed ([out, b])
  so the contraction dim (d_in) sits on SBUF partitions:
    - logits^T = gw^T-chunks  (stationary) x x^T-chunks (moving), PSUM-accum
    - E^T = exp(logits^T + gb) on ScalarE
    - per-leaf row broadcast of E^T via one-hot stationary matmuls
    - xg^T[i,b] = x^T[i,b] * E^T[l,b] on VectorE (the gated activations)
    - out^T[oc] += pw^T[l,i,oc-block] (stationary) x xg^T (moving), one long
      PSUM accumulation per 128-row output chunk (8 banks, 257 matmuls each)
    - final scale by broadcast 1/sum_l E and DMA out.
  All matmuls run in float32r (FP22, 1 PE cycle/row at N=512, ~4x faster
  than fp32) with fp32 PSUM accumulation.
"""
import numpy as np

B, D_IN, D_OUT, L = 4096, 1024, 1024, 32
NCORES = 8
P = 128                 # SBUF partitions
BSH = B // NCORES       # 512 batch rows per core
IC = D_IN // P          # 8 contraction chunks
OC = D_OUT // P         # 8 output chunks

_RUNNER = None


def _round_fp32r(x: np.ndarray) -> np.ndarray:
    """Round fp32 to fp32r (e8m13): RTNE drop of the low 10 mantissa bits."""
    v = np.ascontiguousarray(x, dtype=np.float32).view(np.uint32)
    v = (v + 0x1FF + ((v >> 10) & 1)) & np.uint32(0xFFFFFC00)
    return v.view(np.float32)


def _build_module():
    import concourse.mybir as mybir
    import concourse.tile as tile
    from concourse import bacc
    from concourse.bass import ts

    F32 = mybir.dt.float32
    F32R = mybir.dt.float32r
    BF16 = mybir.dt.bfloat16
    AFT = mybir.ActivationFunctionType

    nc = bacc.Bacc("TRN2", target_bir_lowering=False, debug=False)

    xt = nc.dram_tensor("xt", [D_IN, BSH], F32R, kind="ExternalInput")
    pwt = nc.dram_tensor("pwt", [L, D_IN, D_OUT], BF16, kind="ExternalInput")
    gwt_d = nc.dram_tensor("gwt", [D_IN, L], F32R, kind="ExternalInput")
    gb_d = nc.dram_tensor("gb", [L, 1], F32, kind="ExternalInput")
    pbt_d = nc.dram_tensor("pbt", [L, D_OUT], F32R, kind="ExternalInput")
    ones_d = nc.dram_tensor("ones", [L, P], F32R, kind="ExternalInput")
    outt = nc.dram_tensor("outt", [D_OUT, BSH], F32, kind="ExternalOutput")
    den_d = nc.dram_tensor("den", [1, BSH], F32, kind="ExternalOutput")

    with tile.TileContext(nc) as tc:
        with tc.tile_pool(name="const", bufs=1) as cp:
            xts = cp.tile([P, IC * BSH], F32R, tag="xts")
            gwt = cp.tile([P, IC * L], F32R, tag="gwt")
            gbt = cp.tile([L, 1], F32, tag="gbt")
            pbt = cp.tile([L, D_OUT], F32R, tag="pbt")
            ones = cp.tile([L, P], F32R, tag="ones")
            et = cp.tile([L, BSH], F32R, tag="et")
            dens = cp.tile([1, BSH], F32, tag="dens")
            # one dedicated [1, BSH] tile per leaf: the HW partition-broadcast
            # ucode uses the tile base address (AP offsets are not honored),
            # so each leaf row needs its own tile.
            els = [
                cp.tile([1, BSH], F32, tag=f"el{l}", name=f"el{l}")
                for l in range(L)
            ]

            # input DMAs: tiny warmup/bias constants first so the PE warmup
            # can run while x^T streams in; the sync queue is otherwise
            # reserved for the pw weight stream
            nc.scalar.dma_start(ones[:], ones_d[:])
            nc.scalar.dma_start(gbt[:], gb_d[:])
            nc.scalar.dma_start(
                gwt[:].rearrange("p (c l) -> p c l", l=L),
                gwt_d.rearrange("(c p) l -> p c l", p=P),
            )
            # x^T chunks stay off the sync queue entirely — the eagerly
            # prefetched pw weight stream otherwise delays them (and the
            # whole gating chain) by many microseconds.
            for c in range(IC):
                eng = nc.gpsimd if c % 2 == 0 else nc.scalar
                eng.dma_start(xts[:, ts(c, BSH)], xt[ts(c, P), :])
            nc.scalar.dma_start(pbt[:], pbt_d[:])

            # ---- gating head ----
            # Preload the Exp activation table off the critical path.
            scratch = cp.tile([L, 1], F32, tag="scratch")
            nc.scalar.activation(scratch[:], gbt[:], AFT.Exp, bias=0.0, scale=1.0)

            # memset-sourced warmup weights: no DMA dependency, so the PE
            # warm-up starts as soon as the engines are up.
            warm_w = cp.tile([L, P], BF16, tag="warm_w")
            nc.gpsimd.memset(warm_w[:], 1.0)

            with tc.tile_pool(name="hpsum", bufs=4, space="PSUM") as hp:
                # PE warm-up: tiny matmuls while inputs DMA in, so the HAM
                # clock-gate opens before the real work.
                pw_ps = hp.tile([P, 64], F32, tag="warm")
                for _ in range(14):
                    nc.tensor.matmul(pw_ps[:], warm_w[:], warm_w[:, :64],
                                     start=True, stop=True)
                pg = hp.tile([L, BSH], F32, tag="hp")
                for c in range(IC):
                    nc.tensor.matmul(
                        pg[:],
                        gwt[:, ts(c, L)],
                        xts[:, ts(c, BSH)],
                        start=(c == 0),
                        stop=(c == IC - 1),
                    )
                # leaf 0's gate row computed directly into its dedicated tile
                # (skips the scatter-DMA completion latency on the critical
                # path; engines can only address start-partition 0), then the
                # full E^T = exp(logits^T + gb)
                nc.scalar.activation(
                    els[0][:], pg[0:1, :], AFT.Exp,
                    bias=gbt[0:1, :], scale=1.0,
                )
                nc.scalar.activation(et[:], pg[:], AFT.Exp, bias=gbt[:], scale=1.0)

                # denominator (sum_l E), broadcast to 128 partitions: ones^T @ E^T
                pd = hp.tile([P, BSH], F32, tag="hp")
                nc.tensor.matmul(pd[:], ones[:], et[:], start=True, stop=True)
                nc.scalar.copy(dens[:], pd[0:1, :])
                nc.sync.dma_start(den_d[:], dens[:])

                # scatter E^T rows into the dedicated per-leaf tiles (tiny DMAs
                # spread over both HWDGE queues)
                for l in range(1, L):
                    nc.scalar.dma_start(els[l][:], et[l:l + 1, :].bitcast(F32))

            # ---- main accumulation ----
            with tc.tile_pool(name="opsum", bufs=8, space="PSUM") as op, \
                 tc.tile_pool(name="wpool", bufs=12) as wp, \
                 tc.tile_pool(name="xgpool", bufs=10) as xp, \
                 tc.tile_pool(name="ebcpool", bufs=4) as bp, \
                 tc.tile_pool(name="evac", bufs=4) as ep:
                pos = [
                    op.tile([P, BSH], F32, tag="po", name=f"po{oc}")
                    for oc in range(OC)
                ]
                # bias term starts each accumulation group
                for oc in range(OC):
                    nc.tensor.matmul(
                        pos[oc][:], pbt[:, ts(oc, P)], et[:],
                        start=True, stop=False,
                    )
                def evacuate(oc):
                    # unnormalized sums out (normalization happens on host);
                    # copies alternate ScalarE/VectorE
                    ot = ep.tile([P, BSH], F32, tag="ot", name=f"ot{oc}")
                    if oc % 2 == 0:
                        nc.scalar.copy(ot[:], pos[oc][:])
                        nc.sync.dma_start(outt[ts(oc, P), :], ot[:])
                    else:
                        nc.vector.tensor_copy(ot[:], pos[oc][:])
                        nc.scalar.dma_start(outt[ts(oc, P), :], ot[:])

                for l in range(L - 1):
                    # broadcast this leaf's gates across partitions on GpSimd
                    ebc = bp.tile([P, BSH], F32, tag="ebc")
                    nc.gpsimd.partition_broadcast(ebc[:], els[l][:])
                    for c in range(IC):
                        wt = wp.tile([P, D_OUT], BF16, tag="wt")
                        nc.sync.dma_start(wt[:], pwt[l, ts(c, P), :])
                        xg = xp.tile([P, BSH], BF16, tag="xg")
                        nc.vector.tensor_mul(
                            xg[:], xts[:, ts(c, BSH)], ebc[:]
                        )
                        for oc in range(OC):
                            nc.tensor.matmul(
                                pos[oc][:], wt[:, ts(oc, P)], xg[:],
                                start=False, stop=False,
                            )
                # Last leaf: bank-at-a-time so 7 of 8 banks finish early and
                # their evacuation + output DMAs overlap the remaining matmuls.
                l = L - 1
                ebc = bp.tile([P, BSH], F32, tag="ebc")
                nc.gpsimd.partition_broadcast(ebc[:], els[l][:])
                wts, xgs = [], []
                for c in range(IC):
                    wt = wp.tile([P, D_OUT], BF16, tag="wt", name=f"wtl{c}")
                    nc.sync.dma_start(wt[:], pwt[l, ts(c, P), :])
                    wts.append(wt)
                    xg = xp.tile([P, BSH], BF16, tag="xg", name=f"xgl{c}")
                    nc.vector.tensor_mul(xg[:], xts[:, ts(c, BSH)], ebc[:])
                    xgs.append(xg)
                for oc in range(OC):
                    for c in range(IC):
                        nc.tensor.matmul(
                            pos[oc][:], wts[c][:, ts(oc, P)], xgs[c][:],
                            start=False, stop=(c == IC - 1),
                        )
                    evacuate(oc)

    nc.compile()
    return nc


def _make_runner(nc):
    """Cached shard_map-jitted executor over 8 cores (mirrors
    concourse.bass2jax.run_bass_via_pjrt, but reusable across calls)."""
    import jax
    import numpy as np
    from jax.sharding import Mesh, PartitionSpec
    from jax.experimental.shard_map import shard_map
    import concourse.mybir as mybir
    from concourse.bass2jax import (
        _bass_exec_p,
        install_neuronx_cc_hook,
        partition_id_tensor,
    )

    install_neuronx_cc_hook()

    partition_name = (
        nc.partition_id_tensor.name if nc.partition_id_tensor else None
    )
    in_names, out_names, out_avals, zero_shapes = [], [], [], []
    for alloc in nc.m.functions[0].allocations:
        if not isinstance(alloc, mybir.MemoryLocationSet):
            continue
        name = alloc.memorylocations[0].name
        if alloc.kind == "ExternalInput":
            if name != partition_name:
                in_names.append(name)
        elif alloc.kind == "ExternalOutput":
            shape = tuple(alloc.tensor_shape)
            dtype = mybir.dt.np(alloc.dtype)
            out_avals.append(jax.core.ShapedArray(shape, dtype))
            zero_shapes.append((shape, dtype))
            out_names.append(name)
    n_params = len(in_names)
    n_outs = len(out_avals)
    all_names = tuple(in_names + out_names)
    if partition_name is not None:
        all_names = all_names + (partition_name,)
    donate = tuple(range(n_params, n_params + n_outs))

    def _body(*args):
        operands = list(args)
        if partition_name is not None:
            operands.append(partition_id_tensor())
        outs = _bass_exec_p.bind(
            *operands,
            out_avals=tuple(out_avals),
            in_names=all_names,
            out_names=tuple(out_names),
            lowering_input_output_aliases=(),
            sim_require_finite=True,
            sim_require_nnan=True,
            nc=nc,
        )
        return tuple(outs)

    devices = jax.devices()[:NCORES]
    mesh = Mesh(np.asarray(devices), ("core",))
    sharded = jax.jit(
        shard_map(
            _body,
            mesh=mesh,
            in_specs=(PartitionSpec("core"),) * (n_params + n_outs),
            out_specs=(PartitionSpec("core"),) * n_outs,
            check_rep=False,
        ),
        donate_argnums=donate,
        keep_unused=True,
    )

    def run(in_maps):
        concat_in = [
            np.concatenate([m[name] for m in in_maps], axis=0)
            for name in in_names
        ]
        concat_zeros = [
            np.zeros((NCORES * s[0], *s[1:]), dt) for s, dt in zero_shapes
        ]
        out_arrs = sharded(*concat_in, *concat_zeros)
        return [
            {
                name: np.asarray(out_arrs[i]).reshape(
                    NCORES, *out_avals[i].shape
                )[c]
                for i, name in enumerate(out_names)
            }
            for c in range(NCORES)
        ]

    return run


def make_in_maps(x, gw, gb, pw, pb):
    """Shard + lay out the full inputs into per-core input maps."""
    import ml_dtypes
    xr = _round_fp32r(x)
    pwt = np.ascontiguousarray(
        pw.transpose(2, 1, 0).astype(ml_dtypes.bfloat16))            # [L, D_IN, D_OUT]
    gwr = _round_fp32r(gw)
    pbt = _round_fp32r(np.ascontiguousarray(pb.T))                    # [L, D_OUT]
    gbc = np.ascontiguousarray(gb, dtype=np.float32).reshape(L, 1)
    in_maps = []
    for c in range(NCORES):
        xt = np.ascontiguousarray(xr[c * BSH:(c + 1) * BSH, :].T)     # [D_IN, BSH]
        in_maps.append({
            "xt": xt, "pwt": pwt, "gwt": gwr, "gb": gbc,
            "pbt": pbt,
            "ones": np.ones((L, P), dtype=np.float32),
        })
    return in_maps


def _get_runner():
    global _RUNNER
    if _RUNNER is None:
        nc = _build_module()
        try:
            _RUNNER = _make_runner(nc)
        except Exception:
            # Fallback: the (slower, non-cached) stock execution path.
            from concourse.bass_utils import run_bass_kernel_spmd

            def _run(in_maps):
                return run_bass_kernel_spmd(
                    nc, in_maps, core_ids=list(range(NCORES))
                ).results

            _RUNNER = _run
    return _RUNNER


def kernel(x, gw, gb, pw, pb):
    global _RUNNER
    in_maps = make_in_maps(x, gw, gb, pw, pb)
    try:
        results = _get_runner()(in_maps)
    except Exception:
        # One retry with a freshly built runner (e.g. transient device error).
        _RUNNER = None
        results = _get_runner()(in_maps)
    out = np.concatenate(
        [r["outt"].T / r["den"].reshape(BSH, 1) for r in results], axis=0
    )
    return np.ascontiguousarray(out, dtype=np.float32)


# revision 52
# speedup vs baseline: 1.0065x; 1.0065x over previous
"""Trainium2 Bass kernel for soft-MoE routing:
    gatings = softmax(x @ gw + gb, axis=1)            # [B, L]
    proj    = einsum('bi,oil->bol', x, pw)            # [B, D_OUT, L]
    result  = einsum('bol,bl->bo', proj, gatings) + gatings @ pb.T

Strategy (data-parallel over batch, 8 NeuronCores, 512 rows each):
  result[b,o] = ( sum_l E[b,l] * (x @ W_l^T + pb_l)[b,o] ) / sum_l E[b,l]
  with E = exp(x@gw + gb) (unnormalized; normalization folded into a final
  elementwise scale). Per core, everything is computed transposed ([out, b])
  so the contraction dim (d_in) sits on SBUF partitions:
    - logits^T = gw^T-chunks  (stationary) x x^T-chunks (moving), PSUM-accum
    - E^T = exp(logits^T + gb) on ScalarE
    - per-leaf row broadcast of E^T via one-hot stationary matmuls
    - xg^T[i,b] = x^T[i,b] * E^T[l,b] on VectorE (the gated activations)
    - out^T[oc] += pw^T[l,i,oc-block] (stationary) x xg^T (moving), one long
      PSUM accumulation per 128-row output chunk (8 banks, 257 matmuls each)
    - final scale by broadcast 1/sum_l E and DMA out.
  All matmuls run in float32r (FP22, 1 PE cycle/row at N=512, ~4x faster
  than fp32) with fp32 PSUM accumulation.
"""
import numpy as np

B, D_IN, D_OUT, L = 4096, 1024, 1024, 32
NCORES = 8
P = 128                 # SBUF partitions
BSH = B // NCORES       # 512 batch rows per core
IC = D_IN // P          # 8 contraction chunks
OC = D_OUT // P         # 8 output chunks

_RUNNER = None


def _round_fp32r(x: np.ndarray) -> np.ndarray:
    """Round fp32 to fp32r (e8m13): RTNE drop of the low 10 mantissa bits."""
    v = np.ascontiguousarray(x, dtype=np.float32).view(np.uint32)
    v = (v + 0x1FF + ((v >> 10) & 1)) & np.uint32(0xFFFFFC00)
    return v.view(np.float32)


def _build_module():
    import concourse.mybir as mybir
    import concourse.tile as tile
    from concourse import bacc
    from concourse.bass import ts

    F32 = mybir.dt.float32
    F32R = mybir.dt.float32r
    BF16 = mybir.dt.bfloat16
    AFT = mybir.ActivationFunctionType

    nc = bacc.Bacc("TRN2", target_bir_lowering=False, debug=False)

    xt = nc.dram_tensor("xt", [D_IN, BSH], F32R, kind="ExternalInput")
    pwt = nc.dram_tensor("pwt", [L, D_IN, D_OUT], BF16, kind="ExternalInput")
    gwt_d = nc.dram_tensor("gwt", [D_IN, L], F32R, kind="ExternalInput")
    gb_d = nc.dram_tensor("gb", [L, 1], F32, kind="ExternalInput")
    pbt_d = nc.dram_tensor("pbt", [L, D_OUT], F32R, kind="ExternalInput")
    ones_d = nc.dram_tensor("ones", [L, P], F32R, kind="ExternalInput")
    outt = nc.dram_tensor("outt", [D_OUT, BSH], F32, kind="ExternalOutput")
    den_d = nc.dram_tensor("den", [1, BSH], F32, kind="ExternalOutput")

    with tile.TileContext(nc) as tc:
        with tc.tile_pool(name="const", bufs=1) as cp:
            xts = cp.tile([P, IC * BSH], F32R, tag="xts")
            gwt = cp.tile([P, IC * L], F32R, tag="gwt")
            gbt = cp.tile([L, 1], F32, tag="gbt")
            pbt = cp.tile([L, D_OUT], F32R, tag="pbt")
            ones = cp.tile([L, P], F32R, tag="ones")
            et = cp.tile([L, BSH], F32R, tag="et")
            dens = cp.tile([1, BSH], F32, tag="dens")
            # one dedicated [1, BSH] tile per leaf: the HW partition-broadcast
            # ucode uses the tile base address (AP offsets are not honored),
            # so each leaf row needs its own tile.
            els = [
                cp.tile([1, BSH], F32, tag=f"el{l}", name=f"el{l}")
                for l in range(L)
            ]

            # input DMAs: tiny warmup/bias constants first so the PE warmup
            # can run while x^T streams in; the sync queue is otherwise
            # reserved for the pw weight stream
            nc.scalar.dma_start(ones[:], ones_d[:])
            nc.scalar.dma_start(gbt[:], gb_d[:])
            nc.scalar.dma_start(
                gwt[:].rearrange("p (c l) -> p c l", l=L),
                gwt_d.rearrange("(c p) l -> p c l", p=P),
            )
            for c in range(IC):
                if c >= 6:
                    eng = nc.gpsimd      # third DMA channel (SWDGE)
                else:
                    eng = nc.sync if c % 2 == 0 else nc.scalar
                eng.dma_start(xts[:, ts(c, BSH)], xt[ts(c, P), :])
            nc.scalar.dma_start(pbt[:], pbt_d[:])

            # ---- gating head ----
            # Preload the Exp activation table off the critical path.
            scratch = cp.tile([L, 1], F32, tag="scratch")
            nc.scalar.activation(scratch[:], gbt[:], AFT.Exp, bias=0.0, scale=1.0)

            # memset-sourced warmup weights: no DMA dependency, so the PE
            # warm-up starts as soon as the engines are up.
            warm_w = cp.tile([L, P], BF16, tag="warm_w")
            nc.gpsimd.memset(warm_w[:], 1.0)

            with tc.tile_pool(name="hpsum", bufs=4, space="PSUM") as hp:
                # PE warm-up: tiny matmuls while inputs DMA in, so the HAM
                # clock-gate opens before the real work.
                pw_ps = hp.tile([P, 64], F32, tag="warm")
                for _ in range(14):
                    nc.tensor.matmul(pw_ps[:], warm_w[:], warm_w[:, :64],
                                     start=True, stop=True)
                pg = hp.tile([L, BSH], F32, tag="hp")
                for c in range(IC):
                    nc.tensor.matmul(
                        pg[:],
                        gwt[:, ts(c, L)],
                        xts[:, ts(c, BSH)],
                        start=(c == 0),
                        stop=(c == IC - 1),
                    )
                # E^T = exp(logits^T + gb)
                nc.scalar.activation(et[:], pg[:], AFT.Exp, bias=gbt[:], scale=1.0)

                # denominator (sum_l E), broadcast to 128 partitions: ones^T @ E^T
                pd = hp.tile([P, BSH], F32, tag="hp")
                nc.tensor.matmul(pd[:], ones[:], et[:], start=True, stop=True)
                nc.scalar.copy(dens[:], pd[0:1, :])
                nc.sync.dma_start(den_d[:], dens[:])

                # scatter E^T rows into the dedicated per-leaf tiles (tiny DMAs
                # spread over both HWDGE queues)
                for l in range(L):
                    nc.scalar.dma_start(els[l][:], et[l:l + 1, :].bitcast(F32))

            # ---- main accumulation ----
            with tc.tile_pool(name="opsum", bufs=8, space="PSUM") as op, \
                 tc.tile_pool(name="wpool", bufs=12) as wp, \
                 tc.tile_pool(name="xgpool", bufs=10) as xp, \
                 tc.tile_pool(name="ebcpool", bufs=4) as bp, \
                 tc.tile_pool(name="evac", bufs=4) as ep:
                pos = [
                    op.tile([P, BSH], F32, tag="po", name=f"po{oc}")
                    for oc in range(OC)
                ]
                # bias term starts each accumulation group
                for oc in range(OC):
                    nc.tensor.matmul(
                        pos[oc][:], pbt[:, ts(oc, P)], et[:],
                        start=True, stop=False,
                    )
                def evacuate(oc):
                    # unnormalized sums out (normalization happens on host);
                    # copies alternate ScalarE/VectorE
                    ot = ep.tile([P, BSH], F32, tag="ot", name=f"ot{oc}")
                    if oc % 2 == 0:
                        nc.scalar.copy(ot[:], pos[oc][:])
                        nc.sync.dma_start(outt[ts(oc, P), :], ot[:])
                    else:
                        nc.vector.tensor_copy(ot[:], pos[oc][:])
                        nc.scalar.dma_start(outt[ts(oc, P), :], ot[:])

                for l in range(L - 1):
                    # broadcast this leaf's gates across partitions on GpSimd
                    ebc = bp.tile([P, BSH], F32, tag="ebc")
                    nc.gpsimd.partition_broadcast(ebc[:], els[l][:])
                    for c in range(IC):
                        wt = wp.tile([P, D_OUT], BF16, tag="wt")
                        nc.sync.dma_start(wt[:], pwt[l, ts(c, P), :])
                        xg = xp.tile([P, BSH], BF16, tag="xg")
                        nc.vector.tensor_mul(
                            xg[:], xts[:, ts(c, BSH)], ebc[:]
                        )
                        for oc in range(OC):
                            nc.tensor.matmul(
                                pos[oc][:], wt[:, ts(oc, P)], xg[:],
                                start=False, stop=False,
                            )
                # Last leaf: bank-at-a-time so 7 of 8 banks finish early and
                # their evacuation + output DMAs overlap the remaining matmuls.
                l = L - 1
                ebc = bp.tile([P, BSH], F32, tag="ebc")
                nc.gpsimd.partition_broadcast(ebc[:], els[l][:])
                wts, xgs = [], []
                for c in range(IC):
                    wt = wp.tile([P, D_OUT], BF16, tag="wt", name=f"wtl{c}")
                    nc.sync.dma_start(wt[:], pwt[l, ts(c, P), :])
                    wts.append(wt)
                    xg = xp.tile([P, BSH], BF16, tag="xg", name=f"xgl{c}")
                    nc.vector.tensor_mul(xg[:], xts[:, ts(c, BSH)], ebc[:])
                    xgs.append(xg)
                for oc in range(OC):
                    for c in range(IC):
                        nc.tensor.matmul(
                            pos[oc][:], wts[c][:, ts(oc, P)], xgs[c][:],
                            start=False, stop=(c == IC - 1),
                        )
                    evacuate(oc)

    nc.compile()
    return nc


def _make_runner(nc):
    """Cached shard_map-jitted executor over 8 cores (mirrors
    concourse.bass2jax.run_bass_via_pjrt, but reusable across calls)."""
    import jax
    import numpy as np
    from jax.sharding import Mesh, PartitionSpec
    from jax.experimental.shard_map import shard_map
    import concourse.mybir as mybir
    from concourse.bass2jax import (
        _bass_exec_p,
        install_neuronx_cc_hook,
        partition_id_tensor,
    )

    install_neuronx_cc_hook()

    partition_name = (
        nc.partition_id_tensor.name if nc.partition_id_tensor else None
    )
    in_names, out_names, out_avals, zero_shapes = [], [], [], []
    for alloc in nc.m.functions[0].allocations:
        if not isinstance(alloc, mybir.MemoryLocationSet):
            continue
        name = alloc.memorylocations[0].name
        if alloc.kind == "ExternalInput":
            if name != partition_name:
                in_names.append(name)
        elif alloc.kind == "ExternalOutput":
            shape = tuple(alloc.tensor_shape)
            dtype = mybir.dt.np(alloc.dtype)
            out_avals.append(jax.core.ShapedArray(shape, dtype))
            zero_shapes.append((shape, dtype))
            out_names.append(name)
    n_params = len(in_names)
    n_outs = len(out_avals)
    all_names = tuple(in_names + out_names)
    if partition_name is not None:
        all_names = all_names + (partition_name,)
    donate = tuple(range(n_params, n_params + n_outs))

    def _body(*args):
        operands = list(args)
        if partition_name is not None:
            operands.append(partition_id_tensor())
        outs = _bass_exec_p.bind(
            *operands,
            out_avals=tuple(out_avals),
            in_names=all_names,
            out_names=tuple(out_names),
            lowering_input_output_aliases=(),
            sim_require_finite=True,
            sim_require_nnan=True,
            nc=nc,
        )
        return tuple(outs)

    devices = jax.devices()[:NCORES]
    mesh = Mesh(np.asarray(devices), ("core",))
    sharded = jax.jit(
        shard_map(
            _body,
            mesh=mesh,
            in_specs=(PartitionSpec("core"),) * (n_params + n_outs),
            out_specs=(PartitionSpec("core"),) * n_outs,
            check_rep=False,
        ),
        donate_argnums=donate,
        keep_unused=True,
    )

    def run(in_maps):
        concat_in = [
            np.concatenate([m[name] for m in in_maps], axis=0)
            for name in in_names
        ]
        concat_zeros = [
            np.zeros((NCORES * s[0], *s[1:]), dt) for s, dt in zero_shapes
        ]
        out_arrs = sharded(*concat_in, *concat_zeros)
        return [
            {
                name: np.asarray(out_arrs[i]).reshape(
                    NCORES, *out_avals[i].shape
                )[c]
                for i, name in enumerate(out_names)
            }
            for c in range(NCORES)
        ]

    return run


def make_in_maps(x, gw, gb, pw, pb):
    """Shard + lay out the full inputs into per-core input maps."""
    import ml_dtypes
    xr = _round_fp32r(x)
    pwt = np.ascontiguousarray(
        pw.transpose(2, 1, 0).astype(ml_dtypes.bfloat16))            # [L, D_IN, D_OUT]
    gwr = _round_fp32r(gw)
    pbt = _round_fp32r(np.ascontiguousarray(pb.T))                    # [L, D_OUT]
    gbc = np.ascontiguousarray(gb, dtype=np.float32).reshape(L, 1)
    in_maps = []
    for c in range(NCORES):
        xt = np.ascontiguousarray(xr[c * BSH:(c + 1) * BSH, :].T)     # [D_IN, BSH]
        in_maps.append({
            "xt": xt, "pwt": pwt, "gwt": gwr, "gb": gbc,
            "pbt": pbt,
            "ones": np.ones((L, P), dtype=np.float32),
        })
    return in_maps


def _get_runner():
    global _RUNNER
    if _RUNNER is None:
        nc = _build_module()
        try:
            _RUNNER = _make_runner(nc)
        except Exception:
            # Fallback: the (slower, non-cached) stock execution path.
            from concourse.bass_utils import run_bass_kernel_spmd

            def _run(in_maps):
                return run_bass_kernel_spmd(
                    nc, in_maps, core_ids=list(range(NCORES))
                ).results

            _RUNNER = _run
    return _RUNNER


def kernel(x, gw, gb, pw, pb):
    global _RUNNER
    in_maps = make_in_maps(x, gw, gb, pw, pb)
    try:
        results = _get_runner()(in_maps)
    except Exception:
        # One retry with a freshly built runner (e.g. transient device error).
        _RUNNER = None
        results = _get_runner()(in_maps)
    out = np.concatenate(
        [r["outt"].T / r["den"].reshape(BSH, 1) for r in results], axis=0
    )
    return np.ascontiguousarray(out, dtype=np.float32)
